# revision 7
# baseline (speedup 1.0000x reference)
"""Self-contained segment-max kernel for 8 TRN2 NeuronCores.

out[s, d] = max over rows i with index[i] == s of x[i, d]; empty
segments yield 0. Shapes hardcoded per the problem spec:
x [4194304, 64] f32, index [4194304] int64 (sorted), 65536 segments.

Algorithm (output-stationary chunked gather):
  * Host: from the sorted index, bincount/cumsum give per-segment row
    ranges [a_s, b_s). Segments are split into 8 contiguous ranges with
    ~equal row counts; segments never straddle cores, so there is no
    cross-core combine.
  * Per core, rows are covered by fixed windows every W_STRIDE=32512
    rows so that row offsets relative to a window start fit in int16
    (dma_gather's index dtype; the N_W=32768 AP extent leaves slack for
    segments that start near the window end). Segments belong to the
    window containing their first row.
  * Within a window, non-empty segments are sorted by length (desc) and
    packed into tiles of 128 (one segment per SBUF partition). Each
    segment in a tile is covered by ITER chunks of C consecutive rows,
    chunk j starting at clamp(a + j*C, a, b-C). Duplicate rows from
    clamping are harmless for max; C <= min length in the tile, so
    chunks never cross segment boundaries.
  * Device: per tile-job, dma_gather (elem_step=64 elements = one 256B
    row, elem_size=C*64) pulls 128*ITER chunks into an SBUF tile
    [128, ITER*C*64]; one DVE tensor_reduce(max) over the [p][64][rows]
    strided view produces [128, 64]; DMA to the job's output slot.
    Calls are capped at 8*128 indices each: the runtime's per-lane
    SWDGE descriptor ring holds ~65 descriptors and one gather call
    generates num_idxs/16+1 per lane (larger calls hang the Q7 DGE).
  * Job constants (window, C, ITER) are maxed across cores so a single
    SPMD NEFF serves all 8 cores; all per-core variation lives in the
    gather-index input tensor. Host scatters per-slot results back to
    segment ids and leaves empty segments at 0.

The result is exact (max is order- and duplicate-invariant).
"""

import os
import sys

sys.path.insert(0, "/opt/trn_rl_repo")

import numpy as np

N_FULL = 4194304
NUM_SEGMENTS = 65536
D = 64
N_CORES = 8
W_STRIDE = 32512
N_W = 32768
P = 128
C_DEFAULT = 16
MAX_IT_PER_CALL = 8

LAST_RUN_INFO = {}


def _plan(index, c_default=C_DEFAULT):
    n = index.shape[0]
    counts = np.bincount(index, minlength=NUM_SEGMENTS).astype(np.int64)
    starts = np.zeros(NUM_SEGMENTS + 1, dtype=np.int64)
    np.cumsum(counts, out=starts[1:])
    assert starts[-1] == n

    seg_bounds = np.searchsorted(
        starts, [n * c // N_CORES for c in range(N_CORES + 1)]
    )
    seg_bounds[0] = 0
    seg_bounds[-1] = NUM_SEGMENTS

    core_row0 = [int(starts[seg_bounds[c]]) for c in range(N_CORES)]
    core_rows = [
        int(starts[seg_bounds[c + 1]] - starts[seg_bounds[c]])
        for c in range(N_CORES)
    ]
    max_rows = max(core_rows)
    n_windows = max(1, -(-max_rows // W_STRIDE))
    ns = (n_windows - 1) * W_STRIDE + N_W + 256

    win_segs = [[None] * n_windows for _ in range(N_CORES)]
    for c in range(N_CORES):
        s0, s1 = seg_bounds[c], seg_bounds[c + 1]
        segs = np.arange(s0, s1)[counts[s0:s1] > 0]
        a_rel = starts[segs] - core_row0[c]
        w_of = a_rel // W_STRIDE
        lens = counts[segs]
        assert lens.max(initial=0) <= 256, "segment too long for window slack"
        for w in range(n_windows):
            m = w_of == w
            ss = segs[m]
            order = np.argsort(-lens[m], kind="stable")
            win_segs[c][w] = ss[order]

    jobs = []  # (window, tile, C, ITER) shared across all cores
    for w in range(n_windows):
        t_w = max(-(-len(win_segs[c][w]) // P) for c in range(N_CORES))
        for t in range(t_w):
            minlen = c_default
            maxlen = 1
            for c in range(N_CORES):
                sl = win_segs[c][w][t * P : (t + 1) * P]
                if len(sl):
                    minlen = min(minlen, int(counts[sl].min()))
                    maxlen = max(maxlen, int(counts[sl].max()))
            cc = max(1, min(c_default, minlen))
            it = max(1, -(-maxlen // cc))
            jobs.append((w, t, cc, it))

    njobs = len(jobs)
    idxw = sum(8 * it for (_, _, _, it) in jobs)

    gidx = np.zeros((N_CORES, P, idxw), dtype=np.int16)
    slotseg = np.full((N_CORES, njobs * P), -1, dtype=np.int64)
    for c in range(N_CORES):
        off = 0
        for k, (w, t, cc, it) in enumerate(jobs):
            sl = win_segs[c][w][t * P : (t + 1) * P]
            nsl = len(sl)
            a = np.full(P, w * W_STRIDE, dtype=np.int64)
            b = a + cc
            if nsl:
                a[:nsl] = starts[sl] - core_row0[c]
                b[:nsl] = starts[sl + 1] - core_row0[c]
                slotseg[c, k * P : k * P + nsl] = sl
            j = np.arange(it, dtype=np.int64)[:, None]
            st = np.minimum(a[None, :] + j * cc, b[None, :] - cc)
            st = np.maximum(st, a[None, :]) - w * W_STRIDE
            assert st.min() >= 0 and st.max() < N_W
            # flat order i = j*128 + p matches the gather's dst[p, j]
            flat = st.astype(np.int16).reshape(-1)
            wrapped = flat.reshape(-1, 16).T  # [16, 8*it] idx stream
            gidx[c, :, off : off + 8 * it] = np.tile(wrapped, (8, 1))
            off += 8 * it

    return dict(
        ns=ns,
        idxw=idxw,
        jobs=jobs,
        njobs=njobs,
        gidx=gidx,
        slotseg=slotseg,
        core_row0=core_row0,
        core_rows=core_rows,
    )


def _build(pl, enable_asserts=False, reps=1, queues=1, skip_reduce=False, bufs=6):
    import concourse.bacc as bacc
    import concourse.bass as bass
    import concourse.mybir as mybir
    import concourse.tile as tile

    nc = bacc.Bacc(
        "TRN2",
        debug=False,
        enable_asserts=enable_asserts,
        target_bir_lowering=False,
        num_devices=N_CORES,
        num_swdge_queues=queues,
    )
    xs = nc.dram_tensor("xs", [pl["ns"], D], mybir.dt.float32, kind="ExternalInput")
    gi = nc.dram_tensor(
        "gidx", [P, pl["idxw"]], mybir.dt.int16, kind="ExternalInput"
    )
    out = nc.dram_tensor(
        "out", [pl["njobs"] * P, D], mybir.dt.float32, kind="ExternalOutput"
    )

    with tile.TileContext(nc) as tc:
        with (
            tc.tile_pool(name="idxp", bufs=1) as idxp,
            tc.tile_pool(name="gath", bufs=bufs) as gath,
            tc.tile_pool(name="accp", bufs=3) as accp,
        ):
            gsb = idxp.tile([P, pl["idxw"]], mybir.dt.int16)
            nc.sync.dma_start(out=gsb[:], in_=gi.ap())
            for _rep in range(reps):  # reps>1 only for slope timing
                _build_jobs(nc, bass, mybir, pl, xs, gsb, gath, accp, out, queues, skip_reduce)
    nc.compile()
    return nc


def _build_jobs(nc, bass, mybir, pl, xs, gsb, gath, accp, out, queues=1, skip_reduce=False):
    off = 0
    for k, (w, t, cc, it) in enumerate(pl["jobs"]):
        g = gath.tile([P, it * cc * D], mybir.dt.float32, tag="g")
        in_ap = bass.AP(
            tensor=xs,
            offset=w * W_STRIDE * D,
            ap=[[D, N_W], [1, cc * D]],
        )
        it0 = 0
        while it0 < it:
            itn = min(MAX_IT_PER_CALL, it - it0)
            gv = g[:, it0 * cc * D : (it0 + itn) * cc * D]
            nc.gpsimd.dma_gather(
                gv.rearrange("p (i e) -> p i e", e=cc * D),
                in_ap,
                gsb[:, off + 8 * it0 : off + 8 * (it0 + itn)],
                num_idxs=P * itn,
                num_idxs_reg=P * itn,
                elem_size=cc * D,
                elem_step=D,
                queue_num=k % queues,
            )
            it0 += itn
        if skip_reduce:
            nc.sync.dma_start(out=out[k * P : (k + 1) * P, :], in_=g[:, :D])
        else:
            acc = accp.tile([P, D], mybir.dt.float32, tag="a")
            nc.vector.tensor_reduce(
                acc[:],
                g[:].rearrange("p (r d) -> p d r", d=D),
                axis=mybir.AxisListType.X,
                op=mybir.AluOpType.max,
            )
            nc.sync.dma_start(out=out[k * P : (k + 1) * P, :], in_=acc[:])
        off += 8 * it


def stage_in_maps(x, pl):
    in_maps = []
    for c in range(N_CORES):
        xsh = np.zeros((pl["ns"], D), dtype=np.float32)
        r0, nr = pl["core_row0"][c], pl["core_rows"][c]
        xsh[:nr] = x[r0 : r0 + nr]
        in_maps.append({"xs": xsh, "gidx": np.ascontiguousarray(pl["gidx"][c])})
    return in_maps


def assemble(core_outs, pl):
    out = np.zeros((NUM_SEGMENTS, D), dtype=np.float32)
    for c in range(N_CORES):
        r = np.asarray(core_outs[c]).reshape(-1, D)
        ss = pl["slotseg"][c]
        m = ss >= 0
        out[ss[m]] = r[m]
    return out


def kernel(x, index):
    from concourse.bass_utils import run_bass_kernel_spmd

    x = np.ascontiguousarray(np.asarray(x, dtype=np.float32))
    index = np.asarray(index)
    assert x.shape == (N_FULL, D)

    pl = _plan(index)
    nc = _build(pl)
    in_maps = stage_in_maps(x, pl)

    trace = os.environ.get("SEGKERN_TRACE", "0") == "1"
    res = run_bass_kernel_spmd(
        nc, in_maps, core_ids=list(range(N_CORES)), trace=trace
    )
    LAST_RUN_INFO.clear()
    LAST_RUN_INFO.update(
        exec_time_ns=res.exec_time_ns,
        mean_exec_time_ns=res.mean_exec_time_ns,
        trace=res.instructions_and_trace[1] if res.instructions_and_trace else None,
        profile_json=res.profile_json,
    )
    return assemble([r["out"] for r in res.results], pl)
